# revision 3
# baseline (speedup 1.0000x reference)
"""Trainium2 Bass kernel for MultiModalPromptLearner (embedding_lookup).

Computes, for the full batch B=4096:
  pe       [B, 77, 512]  prompt embeddings: emb-gather with learned-ctx splice
  pt       [B, 77]       constructed prompt token ids (as f32)
  ctx      [4, 512]      passthrough
  proj_ctx [4, 768]      ctx @ proj_w.T + proj_b
  cpt      [8, 4, 512]   passthrough
  vdp      [8, 4, 768]   einsum('knd,kod->kno', cpt, cw) + cb

Sharding: data-parallel over batch across 8 NeuronCores (512 rows each);
emb/ctx/proj/cpt/cw/cb replicated.  The dominant cost is the embedding
gather (645 MB of output), implemented with gpsimd indirect DMA (2 KB row
gathers) pipelined against HWDGE stores.
"""

import sys

if "/opt/trn_rl_repo" not in sys.path:
    sys.path.insert(0, "/opt/trn_rl_repo")

from contextlib import ExitStack

import numpy as np

import concourse.bacc as bacc
import concourse.bass as bass
import concourse.mybir as mybir
import concourse.tile as tile
from concourse.bass import IndirectOffsetOnAxis
from concourse.bass_utils import run_bass_kernel_spmd
from concourse.masks import make_identity

B, L, V, D, NCTX, DEPTH, VD = 4096, 77, 49408, 512, 4, 9, 768
NCORES = 8
BS = B // NCORES  # 512 batch rows per core
NBLK = BS // 128  # 4 partition blocks per core

F32 = mybir.dt.float32
I32 = mybir.dt.int32

# token-slot column groups for the gather:
#   slot 0        -> BOS token embedding
#   slots 1..4    -> learned ctx (no gather; broadcast-written)
#   slots 5..76   -> shifted sentence tokens (incl. EOT fixup at 76)
GROUPS = [(0, 1), (5, 11)] + [(11 * g, 11 * (g + 1)) for g in range(1, 7)]

_CACHE: dict = {}


def _build_program():
    nc = bacc.Bacc("TRN2", target_bir_lowering=False, debug=False)

    text = nc.dram_tensor("text", [BS, L], I32, kind="ExternalInput")
    emb = nc.dram_tensor("emb", [V, D], F32, kind="ExternalInput")
    ctx_t = nc.dram_tensor("ctx", [NCTX, D], F32, kind="ExternalInput")
    pw_t = nc.dram_tensor("proj_w", [VD, D], F32, kind="ExternalInput")
    pb_t = nc.dram_tensor("proj_b", [1, VD], F32, kind="ExternalInput")
    cpt_t = nc.dram_tensor("cpt", [DEPTH - 1, NCTX, D], F32, kind="ExternalInput")
    cw_t = nc.dram_tensor("cw", [DEPTH - 1, VD, D], F32, kind="ExternalInput")
    cb_t = nc.dram_tensor("cb", [DEPTH - 1, VD], F32, kind="ExternalInput")

    pe = nc.dram_tensor("pe", [BS, L, D], F32, kind="ExternalOutput")
    pt = nc.dram_tensor("pt", [BS, L], F32, kind="ExternalOutput")
    pc_t = nc.dram_tensor("proj_ctx", [NCTX, VD], F32, kind="ExternalOutput")
    vdp_t = nc.dram_tensor("vdp", [DEPTH - 1, NCTX, VD], F32, kind="ExternalOutput")

    with tile.TileContext(nc) as tc, ExitStack() as ctx:
        sp = ctx.enter_context(tc.tile_pool(name="small", bufs=2))
        gp = ctx.enter_context(tc.tile_pool(name="gather", bufs=4))
        mp = ctx.enter_context(tc.tile_pool(name="mm", bufs=2))
        pp = ctx.enter_context(tc.tile_pool(name="psum", bufs=2, space="PSUM"))
        cp = ctx.enter_context(tc.tile_pool(name="const", bufs=1))

        # ---- learned ctx -> pe[:, 1:5, :] (DRAM->DRAM broadcast) ----
        ctx_flat = ctx_t[:, :].rearrange("a b -> (a b)")[None, :]
        nc.sync.dma_start(
            out=pe[:, 1:5, :], in_=ctx_flat.to_broadcast((BS, NCTX * D))
        )

        # ---- prompt construction + embedding gather ----
        for blk in range(NBLK):
            b0 = blk * 128
            tt = sp.tile([128, L], I32, tag="text")
            nc.sync.dma_start(out=tt[:], in_=text[b0 : b0 + 128, :])

            ptid = sp.tile([128, L], I32, tag="ptid")
            nc.vector.tensor_copy(out=ptid[:, 0:1], in_=tt[:, 0:1])
            nc.vector.memset(ptid[:, 1:5], 0)
            nc.vector.tensor_copy(out=ptid[:, 5:L], in_=tt[:, 1 : L - NCTX])
            rmax = sp.tile([128, 1], I32, tag="rmax")
            nc.vector.tensor_reduce(
                out=rmax[:], in_=tt[:, :], axis=mybir.AxisListType.X,
                op=mybir.AluOpType.max,
            )
            msk = sp.tile([128, 1], I32, tag="msk")
            nc.vector.tensor_scalar(
                out=msk[:], in0=tt[:, L - NCTX - 1 : L - NCTX], scalar1=1,
                scalar2=None, op0=mybir.AluOpType.min,
            )
            # pt[:, 76] = (text[:, 72] != 0) ? rowmax : 0
            nc.vector.tensor_tensor(
                out=ptid[:, L - 1 : L], in0=rmax[:], in1=msk[:],
                op=mybir.AluOpType.mult,
            )
            ptf = sp.tile([128, L], F32, tag="ptf")
            nc.vector.tensor_copy(out=ptf[:], in_=ptid[:])
            nc.sync.dma_start(out=pt[b0 : b0 + 128, :], in_=ptf[:])

            for lo, hi in GROUPS:
                w = hi - lo
                gt = gp.tile([128, w, D], F32, tag="g")
                for j in range(w):
                    # HW indirect DMA only supports one index per partition
                    nc.gpsimd.indirect_dma_start(
                        out=gt[:, j, :],
                        out_offset=None,
                        in_=emb[:, :],
                        in_offset=IndirectOffsetOnAxis(
                            ap=ptid[:, lo + j : lo + j + 1], axis=0
                        ),
                    )
                nc.sync.dma_start(out=pe[b0 : b0 + 128, lo:hi, :], in_=gt[:])

        # ---- small projections: proj_ctx and vdp ----
        ident = cp.tile([128, 128], F32)
        make_identity(nc, ident[:])

        for dep in range(DEPTH):
            if dep == 0:
                x_dr, w_dr = ctx_t[:, :], pw_t
                b_dr, o_dr = pb_t[:, :], pc_t[:, :]
            else:
                k = dep - 1
                x_dr, w_dr = cpt_t[k], cw_t[k]
                b_dr, o_dr = cb_t[k : k + 1, :], vdp_t[k]

            xs = mp.tile([NCTX, D], F32, tag="xs")
            nc.sync.dma_start(out=xs[:], in_=x_dr)
            xT = mp.tile([128, 4 * NCTX], F32, tag="xT")
            for c in range(4):
                ps = pp.tile([128, NCTX], F32, tag="psT")
                nc.tensor.transpose(
                    out=ps[:], in_=xs[:, c * 128 : (c + 1) * 128],
                    identity=ident[:NCTX, :NCTX],
                )
                nc.vector.tensor_copy(out=xT[:, c * NCTX : (c + 1) * NCTX], in_=ps[:])

            rhs = [
                mp.tile([128, VD], F32, tag=f"rhs{c}", name=f"rhs{c}_{dep}")
                for c in range(4)
            ]
            for m in range(VD // 128):
                wt = mp.tile([128, D], F32, tag="wt")
                nc.sync.dma_start(out=wt[:], in_=w_dr[m * 128 : (m + 1) * 128, :])
                for c in range(4):
                    ps2 = pp.tile([128, 128], F32, tag="psW")
                    nc.tensor.transpose(
                        out=ps2[:], in_=wt[:, c * 128 : (c + 1) * 128],
                        identity=ident[:],
                    )
                    nc.vector.tensor_copy(
                        out=rhs[c][:, m * 128 : (m + 1) * 128], in_=ps2[:]
                    )

            bt = mp.tile([NCTX, VD], F32, tag="bt")
            nc.sync.dma_start(out=bt[:], in_=b_dr.to_broadcast((NCTX, VD)))
            ot = mp.tile([NCTX, VD], F32, tag="ot")
            for n0, wn in ((0, 512), (512, 256)):
                pm = pp.tile([NCTX, 512], F32, tag="pmm")
                for c in range(4):
                    nc.tensor.matmul(
                        pm[:, :wn],
                        lhsT=xT[:, c * NCTX : (c + 1) * NCTX],
                        rhs=rhs[c][:, n0 : n0 + wn],
                        start=(c == 0),
                        stop=(c == 3),
                    )
                nc.vector.tensor_tensor(
                    out=ot[:, n0 : n0 + wn], in0=pm[:, :wn],
                    in1=bt[:, n0 : n0 + wn], op=mybir.AluOpType.add,
                )
            nc.sync.dma_start(out=o_dr, in_=ot[:])

    nc.compile()
    return nc


def kernel(**inputs) -> tuple:
    text = np.ascontiguousarray(np.asarray(inputs["text"]).astype(np.int32))
    emb = np.ascontiguousarray(np.asarray(inputs["emb"], dtype=np.float32))
    ctx = np.ascontiguousarray(np.asarray(inputs["ctx"], dtype=np.float32))
    proj_w = np.ascontiguousarray(np.asarray(inputs["proj_w"], dtype=np.float32))
    proj_b = np.ascontiguousarray(
        np.asarray(inputs["proj_b"], dtype=np.float32).reshape(1, VD)
    )
    cpt = np.ascontiguousarray(np.asarray(inputs["cpt"], dtype=np.float32))
    cw = np.ascontiguousarray(np.asarray(inputs["cw"], dtype=np.float32))
    cb = np.ascontiguousarray(np.asarray(inputs["cb"], dtype=np.float32))

    if "nc" not in _CACHE:
        _CACHE["nc"] = _build_program()
    nc = _CACHE["nc"]

    in_maps = []
    for c in range(NCORES):
        in_maps.append(
            {
                "text": text[c * BS : (c + 1) * BS],
                "emb": emb,
                "ctx": ctx,
                "proj_w": proj_w,
                "proj_b": proj_b,
                "cpt": cpt,
                "cw": cw,
                "cb": cb,
            }
        )

    res = run_bass_kernel_spmd(nc, in_maps, core_ids=list(range(NCORES)))
    _CACHE["last_results"] = res

    pe = np.concatenate([res.results[c]["pe"] for c in range(NCORES)], axis=0)
    pt = np.concatenate([res.results[c]["pt"] for c in range(NCORES)], axis=0)
    proj_ctx = res.results[0]["proj_ctx"]
    vdp = res.results[0]["vdp"]
    return pe, pt, ctx.copy(), proj_ctx, cpt.copy(), vdp


# revision 5
# speedup vs baseline: 1.1210x; 1.1210x over previous
"""Trainium2 Bass kernel for MultiModalPromptLearner (embedding_lookup).

Computes, for the full batch B=4096:
  pe       [B, 77, 512]  prompt embeddings: emb-gather with learned-ctx splice
  pt       [B, 77]       constructed prompt token ids (as f32)
  ctx      [4, 512]      passthrough
  proj_ctx [4, 768]      ctx @ proj_w.T + proj_b
  cpt      [8, 4, 512]   passthrough
  vdp      [8, 4, 768]   einsum('knd,kod->kno', cpt, cw) + cb

Sharding: data-parallel over batch across 8 NeuronCores (512 rows each);
emb/ctx/proj/cpt/cw/cb replicated.  The dominant cost is the embedding
gather (645 MB of output), implemented with gpsimd indirect DMA (2 KB row
gathers) pipelined against HWDGE stores.
"""

import sys

if "/opt/trn_rl_repo" not in sys.path:
    sys.path.insert(0, "/opt/trn_rl_repo")

from contextlib import ExitStack

import numpy as np

import concourse.bacc as bacc
import concourse.bass as bass
import concourse.mybir as mybir
import concourse.tile as tile
from concourse.bass import IndirectOffsetOnAxis
from concourse.bass_utils import run_bass_kernel_spmd
from concourse.masks import make_identity

B, L, V, D, NCTX, DEPTH, VD = 4096, 77, 49408, 512, 4, 9, 768
NCORES = 8
BS = B // NCORES  # 512 batch rows per core
NBLK = BS // 128  # 4 partition blocks per core

F32 = mybir.dt.float32
I32 = mybir.dt.int32

# token-slot column groups for the gather:
#   slot 0        -> BOS token embedding
#   slots 1..4    -> learned ctx (no gather; broadcast-written)
#   slots 5..76   -> shifted sentence tokens (incl. EOT fixup at 76)
GROUPS = [(0, 1), (5, 11)] + [(11 * g, 11 * (g + 1)) for g in range(1, 7)]

_CACHE: dict = {}


def _build_program():
    nc = bacc.Bacc("TRN2", target_bir_lowering=False, debug=False)

    text = nc.dram_tensor("text", [BS, L], I32, kind="ExternalInput")
    emb = nc.dram_tensor("emb", [V, D], F32, kind="ExternalInput")
    ctx_t = nc.dram_tensor("ctx", [NCTX, D], F32, kind="ExternalInput")
    pw_t = nc.dram_tensor("proj_w", [VD, D], F32, kind="ExternalInput")
    pb_t = nc.dram_tensor("proj_b", [1, VD], F32, kind="ExternalInput")
    cpt_t = nc.dram_tensor("cpt", [DEPTH - 1, NCTX, D], F32, kind="ExternalInput")
    cw_t = nc.dram_tensor("cw", [DEPTH - 1, VD, D], F32, kind="ExternalInput")
    cb_t = nc.dram_tensor("cb", [DEPTH - 1, VD], F32, kind="ExternalInput")

    pe = nc.dram_tensor("pe", [BS, L, D], F32, kind="ExternalOutput")
    pt = nc.dram_tensor("pt", [BS, L], F32, kind="ExternalOutput")
    pc_t = nc.dram_tensor("proj_ctx", [NCTX, VD], F32, kind="ExternalOutput")
    vdp_t = nc.dram_tensor("vdp", [DEPTH - 1, NCTX, VD], F32, kind="ExternalOutput")

    with tile.TileContext(nc) as tc, ExitStack() as ctx:
        sp = ctx.enter_context(tc.tile_pool(name="small", bufs=2))
        gp = ctx.enter_context(tc.tile_pool(name="gather", bufs=5))
        mp = ctx.enter_context(tc.tile_pool(name="mm", bufs=2))
        pp = ctx.enter_context(tc.tile_pool(name="psum", bufs=2, space="PSUM"))
        cp = ctx.enter_context(tc.tile_pool(name="const", bufs=1))

        # ---- small projections: proj_ctx and vdp ----
        ident = cp.tile([128, 128], F32)
        make_identity(nc, ident[:])

        for dep in range(DEPTH):
            if dep == 0:
                x_dr, w_dr = ctx_t[:, :], pw_t
                b_dr, o_dr = pb_t[:, :], pc_t[:, :]
            else:
                k = dep - 1
                x_dr, w_dr = cpt_t[k], cw_t[k]
                b_dr, o_dr = cb_t[k : k + 1, :], vdp_t[k]

            xs = mp.tile([NCTX, D], F32, tag="xs")
            nc.scalar.dma_start(out=xs[:], in_=x_dr)
            xT = mp.tile([128, 4 * NCTX], F32, tag="xT")
            for c in range(4):
                ps = pp.tile([128, NCTX], F32, tag="psT")
                nc.tensor.transpose(
                    out=ps[:], in_=xs[:, c * 128 : (c + 1) * 128],
                    identity=ident[:NCTX, :NCTX],
                )
                nc.vector.tensor_copy(out=xT[:, c * NCTX : (c + 1) * NCTX], in_=ps[:])

            rhs = [
                mp.tile([128, VD], F32, tag=f"rhs{c}", name=f"rhs{c}_{dep}")
                for c in range(4)
            ]
            for m in range(VD // 128):
                wt = mp.tile([128, D], F32, tag="wt")
                nc.scalar.dma_start(out=wt[:], in_=w_dr[m * 128 : (m + 1) * 128, :])
                for c in range(4):
                    ps2 = pp.tile([128, 128], F32, tag="psW")
                    nc.tensor.transpose(
                        out=ps2[:], in_=wt[:, c * 128 : (c + 1) * 128],
                        identity=ident[:],
                    )
                    nc.vector.tensor_copy(
                        out=rhs[c][:, m * 128 : (m + 1) * 128], in_=ps2[:]
                    )

            bt = mp.tile([NCTX, VD], F32, tag="bt")
            nc.scalar.dma_start(out=bt[:], in_=b_dr.to_broadcast((NCTX, VD)))
            ot = mp.tile([NCTX, VD], F32, tag="ot")
            for n0, wn in ((0, 512), (512, 256)):
                pm = pp.tile([NCTX, 512], F32, tag="pmm")
                for c in range(4):
                    nc.tensor.matmul(
                        pm[:, :wn],
                        lhsT=xT[:, c * NCTX : (c + 1) * NCTX],
                        rhs=rhs[c][:, n0 : n0 + wn],
                        start=(c == 0),
                        stop=(c == 3),
                    )
                nc.vector.tensor_tensor(
                    out=ot[:, n0 : n0 + wn], in0=pm[:, :wn],
                    in1=bt[:, n0 : n0 + wn], op=mybir.AluOpType.add,
                )
            nc.scalar.dma_start(out=o_dr, in_=ot[:])

        # ---- learned ctx -> pe[:, 1:5, :] (DRAM->DRAM broadcast) ----
        ctx_flat = ctx_t[:, :].rearrange("a b -> (a b)")[None, :]
        nc.scalar.dma_start(
            out=pe[:, 1:5, :], in_=ctx_flat.to_broadcast((BS, NCTX * D))
        )

        # ---- prompt construction + embedding gather ----
        for blk in range(NBLK):
            b0 = blk * 128
            tt = sp.tile([128, L], I32, tag="text")
            nc.scalar.dma_start(out=tt[:], in_=text[b0 : b0 + 128, :])

            ptid = sp.tile([128, L], I32, tag="ptid")
            nc.vector.tensor_copy(out=ptid[:, 0:1], in_=tt[:, 0:1])
            nc.vector.memset(ptid[:, 1:5], 0)
            nc.vector.tensor_copy(out=ptid[:, 5:L], in_=tt[:, 1 : L - NCTX])
            rmax = sp.tile([128, 1], I32, tag="rmax")
            nc.vector.tensor_reduce(
                out=rmax[:], in_=tt[:, :], axis=mybir.AxisListType.X,
                op=mybir.AluOpType.max,
            )
            msk = sp.tile([128, 1], I32, tag="msk")
            nc.vector.tensor_scalar(
                out=msk[:], in0=tt[:, L - NCTX - 1 : L - NCTX], scalar1=1,
                scalar2=None, op0=mybir.AluOpType.min,
            )
            # pt[:, 76] = (text[:, 72] != 0) ? rowmax : 0
            nc.vector.tensor_tensor(
                out=ptid[:, L - 1 : L], in0=rmax[:], in1=msk[:],
                op=mybir.AluOpType.mult,
            )
            ptf = sp.tile([128, L], F32, tag="ptf")
            nc.vector.tensor_copy(out=ptf[:], in_=ptid[:])
            nc.scalar.dma_start(out=pt[b0 : b0 + 128, :], in_=ptf[:])

            for lo, hi in GROUPS:
                w = hi - lo
                gt = gp.tile([128, w, D], F32, tag="g")
                for j in range(w):
                    # HW indirect DMA only supports one index per partition
                    nc.gpsimd.indirect_dma_start(
                        out=gt[:, j, :],
                        out_offset=None,
                        in_=emb[:, :],
                        in_offset=IndirectOffsetOnAxis(
                            ap=ptid[:, lo + j : lo + j + 1], axis=0
                        ),
                    )
                nc.sync.dma_start(out=pe[b0 : b0 + 128, lo:hi, :], in_=gt[:])

    nc.compile()
    return nc


def kernel(**inputs) -> tuple:
    text = np.ascontiguousarray(np.asarray(inputs["text"]).astype(np.int32))
    emb = np.ascontiguousarray(np.asarray(inputs["emb"], dtype=np.float32))
    ctx = np.ascontiguousarray(np.asarray(inputs["ctx"], dtype=np.float32))
    proj_w = np.ascontiguousarray(np.asarray(inputs["proj_w"], dtype=np.float32))
    proj_b = np.ascontiguousarray(
        np.asarray(inputs["proj_b"], dtype=np.float32).reshape(1, VD)
    )
    cpt = np.ascontiguousarray(np.asarray(inputs["cpt"], dtype=np.float32))
    cw = np.ascontiguousarray(np.asarray(inputs["cw"], dtype=np.float32))
    cb = np.ascontiguousarray(np.asarray(inputs["cb"], dtype=np.float32))

    if "nc" not in _CACHE:
        _CACHE["nc"] = _build_program()
    nc = _CACHE["nc"]

    in_maps = []
    for c in range(NCORES):
        in_maps.append(
            {
                "text": text[c * BS : (c + 1) * BS],
                "emb": emb,
                "ctx": ctx,
                "proj_w": proj_w,
                "proj_b": proj_b,
                "cpt": cpt,
                "cw": cw,
                "cb": cb,
            }
        )

    res = run_bass_kernel_spmd(nc, in_maps, core_ids=list(range(NCORES)))
    _CACHE["last_results"] = res

    pe = np.concatenate([res.results[c]["pe"] for c in range(NCORES)], axis=0)
    pt = np.concatenate([res.results[c]["pt"] for c in range(NCORES)], axis=0)
    proj_ctx = res.results[0]["proj_ctx"]
    vdp = res.results[0]["vdp"]
    return pe, pt, ctx.copy(), proj_ctx, cpt.copy(), vdp


# revision 6
# speedup vs baseline: 1.1286x; 1.0068x over previous
"""Trainium2 Bass kernel for MultiModalPromptLearner (embedding_lookup).

Computes, for the full batch B=4096:
  pe       [B, 77, 512]  prompt embeddings: emb-gather with learned-ctx splice
  pt       [B, 77]       constructed prompt token ids (as f32)
  ctx      [4, 512]      passthrough
  proj_ctx [4, 768]      ctx @ proj_w.T + proj_b
  cpt      [8, 4, 512]   passthrough
  vdp      [8, 4, 768]   einsum('knd,kod->kno', cpt, cw) + cb

Sharding: data-parallel over batch across 8 NeuronCores (512 rows each);
emb/ctx/proj/cpt/cw/cb replicated.  The dominant cost is the embedding
gather (645 MB of output), implemented with gpsimd indirect DMA (2 KB row
gathers) pipelined against HWDGE stores.
"""

import sys

if "/opt/trn_rl_repo" not in sys.path:
    sys.path.insert(0, "/opt/trn_rl_repo")

from contextlib import ExitStack

import numpy as np

import concourse.bacc as bacc
import concourse.bass as bass
import concourse.mybir as mybir
import concourse.tile as tile
from concourse.bass import IndirectOffsetOnAxis
from concourse.bass_utils import run_bass_kernel_spmd
from concourse.masks import make_identity

B, L, V, D, NCTX, DEPTH, VD = 4096, 77, 49408, 512, 4, 9, 768
NCORES = 8
BS = B // NCORES  # 512 batch rows per core
NBLK = BS // 128  # 4 partition blocks per core

F32 = mybir.dt.float32
I32 = mybir.dt.int32

# token-slot column groups for the gather:
#   slot 0        -> BOS token embedding
#   slots 1..4    -> learned ctx (no gather; broadcast-written)
#   slots 5..76   -> shifted sentence tokens (incl. EOT fixup at 76)
GROUPS = [(0, 11)] + [(11 * g, 11 * (g + 1)) for g in range(1, 7)]

_CACHE: dict = {}


def _build_program():
    nc = bacc.Bacc("TRN2", target_bir_lowering=False, debug=False)

    text = nc.dram_tensor("text", [BS, L], I32, kind="ExternalInput")
    emb = nc.dram_tensor("emb", [V, D], F32, kind="ExternalInput")
    ctx_t = nc.dram_tensor("ctx", [NCTX, D], F32, kind="ExternalInput")
    pw_t = nc.dram_tensor("proj_w", [VD, D], F32, kind="ExternalInput")
    pb_t = nc.dram_tensor("proj_b", [1, VD], F32, kind="ExternalInput")
    cpt_t = nc.dram_tensor("cpt", [DEPTH - 1, NCTX, D], F32, kind="ExternalInput")
    cw_t = nc.dram_tensor("cw", [DEPTH - 1, VD, D], F32, kind="ExternalInput")
    cb_t = nc.dram_tensor("cb", [DEPTH - 1, VD], F32, kind="ExternalInput")

    pe = nc.dram_tensor("pe", [BS, L, D], F32, kind="ExternalOutput")
    pt = nc.dram_tensor("pt", [BS, L], F32, kind="ExternalOutput")
    pc_t = nc.dram_tensor("proj_ctx", [NCTX, VD], F32, kind="ExternalOutput")
    vdp_t = nc.dram_tensor("vdp", [DEPTH - 1, NCTX, VD], F32, kind="ExternalOutput")

    with tile.TileContext(nc) as tc, ExitStack() as ctx:
        sp = ctx.enter_context(tc.tile_pool(name="small", bufs=2))
        gp = ctx.enter_context(tc.tile_pool(name="gather", bufs=5))
        mp = ctx.enter_context(tc.tile_pool(name="mm", bufs=2))
        pp = ctx.enter_context(tc.tile_pool(name="psum", bufs=2, space="PSUM"))
        cp = ctx.enter_context(tc.tile_pool(name="const", bufs=1))

        # ---- small projections: proj_ctx and vdp ----
        ident = cp.tile([128, 128], F32)
        make_identity(nc, ident[:])

        for dep in range(DEPTH):
            if dep == 0:
                x_dr, w_dr = ctx_t[:, :], pw_t
                b_dr, o_dr = pb_t[:, :], pc_t[:, :]
            else:
                k = dep - 1
                x_dr, w_dr = cpt_t[k], cw_t[k]
                b_dr, o_dr = cb_t[k : k + 1, :], vdp_t[k]

            xs = mp.tile([NCTX, D], F32, tag="xs")
            nc.scalar.dma_start(out=xs[:], in_=x_dr)
            xT = mp.tile([128, 4 * NCTX], F32, tag="xT")
            for c in range(4):
                ps = pp.tile([128, NCTX], F32, tag="psT")
                nc.tensor.transpose(
                    out=ps[:], in_=xs[:, c * 128 : (c + 1) * 128],
                    identity=ident[:NCTX, :NCTX],
                )
                nc.vector.tensor_copy(out=xT[:, c * NCTX : (c + 1) * NCTX], in_=ps[:])

            rhs = [
                mp.tile([128, VD], F32, tag=f"rhs{c}", name=f"rhs{c}_{dep}")
                for c in range(4)
            ]
            for m in range(VD // 128):
                wt = mp.tile([128, D], F32, tag="wt")
                nc.scalar.dma_start(out=wt[:], in_=w_dr[m * 128 : (m + 1) * 128, :])
                for c in range(4):
                    ps2 = pp.tile([128, 128], F32, tag="psW")
                    nc.tensor.transpose(
                        out=ps2[:], in_=wt[:, c * 128 : (c + 1) * 128],
                        identity=ident[:],
                    )
                    nc.vector.tensor_copy(
                        out=rhs[c][:, m * 128 : (m + 1) * 128], in_=ps2[:]
                    )

            bt = mp.tile([NCTX, VD], F32, tag="bt")
            nc.scalar.dma_start(out=bt[:], in_=b_dr.to_broadcast((NCTX, VD)))
            ot = mp.tile([NCTX, VD], F32, tag="ot")
            for n0, wn in ((0, 512), (512, 256)):
                pm = pp.tile([NCTX, 512], F32, tag="pmm")
                for c in range(4):
                    nc.tensor.matmul(
                        pm[:, :wn],
                        lhsT=xT[:, c * NCTX : (c + 1) * NCTX],
                        rhs=rhs[c][:, n0 : n0 + wn],
                        start=(c == 0),
                        stop=(c == 3),
                    )
                nc.vector.tensor_tensor(
                    out=ot[:, n0 : n0 + wn], in0=pm[:, :wn],
                    in1=bt[:, n0 : n0 + wn], op=mybir.AluOpType.add,
                )
            nc.scalar.dma_start(out=o_dr, in_=ot[:])

        # ---- learned ctx, broadcast across partitions (for slots 1..4) ----
        ctx_flat = ctx_t[:, :].rearrange("a b -> (a b)")[None, :]
        ctx_bc = cp.tile([128, NCTX * D], F32)
        nc.scalar.dma_start(out=ctx_bc[:], in_=ctx_flat.to_broadcast((128, NCTX * D)))

        # ---- prompt construction + embedding gather ----
        for blk in range(NBLK):
            b0 = blk * 128
            tt = sp.tile([128, L], I32, tag="text")
            nc.scalar.dma_start(out=tt[:], in_=text[b0 : b0 + 128, :])

            ptid = sp.tile([128, L], I32, tag="ptid")
            nc.vector.tensor_copy(out=ptid[:, 0:1], in_=tt[:, 0:1])
            nc.vector.memset(ptid[:, 1:5], 0)
            nc.vector.tensor_copy(out=ptid[:, 5:L], in_=tt[:, 1 : L - NCTX])
            rmax = sp.tile([128, 1], I32, tag="rmax")
            nc.vector.tensor_reduce(
                out=rmax[:], in_=tt[:, :], axis=mybir.AxisListType.X,
                op=mybir.AluOpType.max,
            )
            msk = sp.tile([128, 1], I32, tag="msk")
            nc.vector.tensor_scalar(
                out=msk[:], in0=tt[:, L - NCTX - 1 : L - NCTX], scalar1=1,
                scalar2=None, op0=mybir.AluOpType.min,
            )
            # pt[:, 76] = (text[:, 72] != 0) ? rowmax : 0
            nc.vector.tensor_tensor(
                out=ptid[:, L - 1 : L], in0=rmax[:], in1=msk[:],
                op=mybir.AluOpType.mult,
            )
            ptf = sp.tile([128, L], F32, tag="ptf")
            nc.vector.tensor_copy(out=ptf[:], in_=ptid[:])
            nc.scalar.dma_start(out=pt[b0 : b0 + 128, :], in_=ptf[:])

            for lo, hi in GROUPS:
                w = hi - lo
                gt = gp.tile([128, w, D], F32, tag="g")
                for j in range(w):
                    l = lo + j
                    if 1 <= l <= 4:
                        continue  # learned-ctx slots, filled below
                    # HW indirect DMA only supports one index per partition
                    nc.gpsimd.indirect_dma_start(
                        out=gt[:, j, :],
                        out_offset=None,
                        in_=emb[:, :],
                        in_offset=IndirectOffsetOnAxis(
                            ap=ptid[:, l : l + 1], axis=0
                        ),
                    )
                if lo == 0:
                    nc.vector.tensor_copy(
                        out=gt[:, 1:5, :], in_=ctx_bc[:].rearrange(
                            "p (n d) -> p n d", n=NCTX
                        )
                    )
                nc.sync.dma_start(out=pe[b0 : b0 + 128, lo:hi, :], in_=gt[:])

    nc.compile()
    return nc


def kernel(**inputs) -> tuple:
    text = np.ascontiguousarray(np.asarray(inputs["text"]).astype(np.int32))
    emb = np.ascontiguousarray(np.asarray(inputs["emb"], dtype=np.float32))
    ctx = np.ascontiguousarray(np.asarray(inputs["ctx"], dtype=np.float32))
    proj_w = np.ascontiguousarray(np.asarray(inputs["proj_w"], dtype=np.float32))
    proj_b = np.ascontiguousarray(
        np.asarray(inputs["proj_b"], dtype=np.float32).reshape(1, VD)
    )
    cpt = np.ascontiguousarray(np.asarray(inputs["cpt"], dtype=np.float32))
    cw = np.ascontiguousarray(np.asarray(inputs["cw"], dtype=np.float32))
    cb = np.ascontiguousarray(np.asarray(inputs["cb"], dtype=np.float32))

    if "nc" not in _CACHE:
        _CACHE["nc"] = _build_program()
    nc = _CACHE["nc"]

    in_maps = []
    for c in range(NCORES):
        in_maps.append(
            {
                "text": text[c * BS : (c + 1) * BS],
                "emb": emb,
                "ctx": ctx,
                "proj_w": proj_w,
                "proj_b": proj_b,
                "cpt": cpt,
                "cw": cw,
                "cb": cb,
            }
        )

    res = run_bass_kernel_spmd(nc, in_maps, core_ids=list(range(NCORES)))
    _CACHE["last_results"] = res

    pe = np.concatenate([res.results[c]["pe"] for c in range(NCORES)], axis=0)
    pt = np.concatenate([res.results[c]["pt"] for c in range(NCORES)], axis=0)
    proj_ctx = res.results[0]["proj_ctx"]
    vdp = res.results[0]["vdp"]
    return pe, pt, ctx.copy(), proj_ctx, cpt.copy(), vdp


# revision 7
# speedup vs baseline: 1.1817x; 1.0470x over previous
"""Trainium2 Bass kernel for MultiModalPromptLearner (embedding_lookup).

Computes, for the full batch B=4096:
  pe       [B, 77, 512]  prompt embeddings: emb-gather with learned-ctx splice
  pt       [B, 77]       constructed prompt token ids (as f32)
  ctx      [4, 512]      passthrough
  proj_ctx [4, 768]      ctx @ proj_w.T + proj_b
  cpt      [8, 4, 512]   passthrough
  vdp      [8, 4, 768]   einsum('knd,kod->kno', cpt, cw) + cb

Sharding: data-parallel over batch across 8 NeuronCores (512 rows each);
emb/ctx/proj/cpt/cw/cb replicated.  The dominant cost is the embedding
gather (645 MB of output), implemented with gpsimd indirect DMA (2 KB row
gathers) pipelined against HWDGE stores.
"""

import sys

if "/opt/trn_rl_repo" not in sys.path:
    sys.path.insert(0, "/opt/trn_rl_repo")

from contextlib import ExitStack

import numpy as np

import concourse.bacc as bacc
import concourse.bass as bass
import concourse.mybir as mybir
import concourse.tile as tile
from concourse.bass import IndirectOffsetOnAxis
from concourse.bass_utils import run_bass_kernel_spmd
from concourse.masks import make_identity

B, L, V, D, NCTX, DEPTH, VD = 4096, 77, 49408, 512, 4, 9, 768
NCORES = 8
BS = B // NCORES  # 512 batch rows per core
NBLK = BS // 128  # 4 partition blocks per core

F32 = mybir.dt.float32
I32 = mybir.dt.int32

# token-slot column groups for the gather:
#   slot 0        -> BOS token embedding
#   slots 1..4    -> learned ctx (no gather; broadcast-written)
#   slots 5..76   -> shifted sentence tokens (incl. EOT fixup at 76)
GROUPS = [(0, 11)] + [(11 * g, 11 * (g + 1)) for g in range(1, 7)]

_CACHE: dict = {}


def _build_program():
    nc = bacc.Bacc("TRN2", target_bir_lowering=False, debug=False)

    text = nc.dram_tensor("text", [BS, L], I32, kind="ExternalInput")
    emb = nc.dram_tensor("emb", [V, D], F32, kind="ExternalInput")
    ctx_t = nc.dram_tensor("ctx", [NCTX, D], F32, kind="ExternalInput")
    pw_t = nc.dram_tensor("proj_w", [VD, D], F32, kind="ExternalInput")
    pb_t = nc.dram_tensor("proj_b", [1, VD], F32, kind="ExternalInput")
    cpt_t = nc.dram_tensor("cpt", [DEPTH - 1, NCTX, D], F32, kind="ExternalInput")
    cw_t = nc.dram_tensor("cw", [DEPTH - 1, VD, D], F32, kind="ExternalInput")
    cb_t = nc.dram_tensor("cb", [DEPTH - 1, VD], F32, kind="ExternalInput")

    pe = nc.dram_tensor("pe", [BS, L, D], F32, kind="ExternalOutput")
    pt = nc.dram_tensor("pt", [BS, L], F32, kind="ExternalOutput")
    pc_t = nc.dram_tensor("proj_ctx", [NCTX, VD], F32, kind="ExternalOutput")
    vdp_t = nc.dram_tensor("vdp", [DEPTH - 1, NCTX, VD], F32, kind="ExternalOutput")

    with tile.TileContext(nc) as tc, ExitStack() as ctx:
        sp = ctx.enter_context(tc.tile_pool(name="small", bufs=2))
        gp = ctx.enter_context(tc.tile_pool(name="gather", bufs=5))
        mp = ctx.enter_context(tc.tile_pool(name="mm", bufs=2))
        pp = ctx.enter_context(tc.tile_pool(name="psum", bufs=2, space="PSUM"))
        cp = ctx.enter_context(tc.tile_pool(name="const", bufs=1))

        # ---- small projections: proj_ctx and vdp ----
        ident = cp.tile([128, 128], F32)
        make_identity(nc, ident[:])

        for dep in range(DEPTH):
            if dep == 0:
                x_dr, w_dr = ctx_t[:, :], pw_t
                b_dr, o_dr = pb_t[:, :], pc_t[:, :]
            else:
                k = dep - 1
                x_dr, w_dr = cpt_t[k], cw_t[k]
                b_dr, o_dr = cb_t[k : k + 1, :], vdp_t[k]

            xs = mp.tile([NCTX, D], F32, tag="xs")
            nc.scalar.dma_start(out=xs[:], in_=x_dr)
            xT = mp.tile([128, 4 * NCTX], F32, tag="xT")
            for c in range(4):
                ps = pp.tile([128, NCTX], F32, tag="psT")
                nc.tensor.transpose(
                    out=ps[:], in_=xs[:, c * 128 : (c + 1) * 128],
                    identity=ident[:NCTX, :NCTX],
                )
                nc.vector.tensor_copy(out=xT[:, c * NCTX : (c + 1) * NCTX], in_=ps[:])

            rhs = [
                mp.tile([128, VD], F32, tag=f"rhs{c}", name=f"rhs{c}_{dep}")
                for c in range(4)
            ]
            for m in range(VD // 128):
                wt = mp.tile([128, D], F32, tag="wt")
                nc.scalar.dma_start(out=wt[:], in_=w_dr[m * 128 : (m + 1) * 128, :])
                for c in range(4):
                    ps2 = pp.tile([128, 128], F32, tag="psW")
                    nc.tensor.transpose(
                        out=ps2[:], in_=wt[:, c * 128 : (c + 1) * 128],
                        identity=ident[:],
                    )
                    nc.vector.tensor_copy(
                        out=rhs[c][:, m * 128 : (m + 1) * 128], in_=ps2[:]
                    )

            bt = mp.tile([NCTX, VD], F32, tag="bt")
            nc.scalar.dma_start(out=bt[:], in_=b_dr.to_broadcast((NCTX, VD)))
            ot = mp.tile([NCTX, VD], F32, tag="ot")
            for n0, wn in ((0, 512), (512, 256)):
                pm = pp.tile([NCTX, 512], F32, tag="pmm")
                for c in range(4):
                    nc.tensor.matmul(
                        pm[:, :wn],
                        lhsT=xT[:, c * NCTX : (c + 1) * NCTX],
                        rhs=rhs[c][:, n0 : n0 + wn],
                        start=(c == 0),
                        stop=(c == 3),
                    )
                nc.vector.tensor_tensor(
                    out=ot[:, n0 : n0 + wn], in0=pm[:, :wn],
                    in1=bt[:, n0 : n0 + wn], op=mybir.AluOpType.add,
                )
            nc.scalar.dma_start(out=o_dr, in_=ot[:])

        # ---- learned ctx, broadcast across partitions (for slots 1..4) ----
        ctx_flat = ctx_t[:, :].rearrange("a b -> (a b)")[None, :]
        ctx_bc = cp.tile([128, NCTX * D], F32)
        nc.scalar.dma_start(out=ctx_bc[:], in_=ctx_flat.to_broadcast((128, NCTX * D)))

        # ---- prompt construction + embedding gather ----
        for blk in range(NBLK):
            b0 = blk * 128
            tt = sp.tile([128, L], I32, tag="text")
            nc.sync.dma_start(out=tt[:], in_=text[b0 : b0 + 128, :])

            ptid = sp.tile([128, L], I32, tag="ptid")
            nc.vector.tensor_copy(out=ptid[:, 0:1], in_=tt[:, 0:1])
            nc.vector.memset(ptid[:, 1:5], 0)
            nc.vector.tensor_copy(out=ptid[:, 5:L], in_=tt[:, 1 : L - NCTX])
            rmax = sp.tile([128, 1], I32, tag="rmax")
            nc.vector.tensor_reduce(
                out=rmax[:], in_=tt[:, :], axis=mybir.AxisListType.X,
                op=mybir.AluOpType.max,
            )
            msk = sp.tile([128, 1], I32, tag="msk")
            nc.vector.tensor_scalar(
                out=msk[:], in0=tt[:, L - NCTX - 1 : L - NCTX], scalar1=1,
                scalar2=None, op0=mybir.AluOpType.min,
            )
            # pt[:, 76] = (text[:, 72] != 0) ? rowmax : 0
            nc.vector.tensor_tensor(
                out=ptid[:, L - 1 : L], in0=rmax[:], in1=msk[:],
                op=mybir.AluOpType.mult,
            )
            ptf = sp.tile([128, L], F32, tag="ptf")
            nc.vector.tensor_copy(out=ptf[:], in_=ptid[:])
            nc.scalar.dma_start(out=pt[b0 : b0 + 128, :], in_=ptf[:])

            for lo, hi in GROUPS:
                w = hi - lo
                gt = gp.tile([128, w, D], F32, tag="g")
                for j in range(w):
                    l = lo + j
                    if 1 <= l <= 4:
                        continue  # learned-ctx slots, filled below
                    # HW indirect DMA only supports one index per partition
                    nc.gpsimd.indirect_dma_start(
                        out=gt[:, j, :],
                        out_offset=None,
                        in_=emb[:, :],
                        in_offset=IndirectOffsetOnAxis(
                            ap=ptid[:, l : l + 1], axis=0
                        ),
                    )
                if lo == 0:
                    nc.vector.tensor_copy(
                        out=gt[:, 1:5, :], in_=ctx_bc[:].rearrange(
                            "p (n d) -> p n d", n=NCTX
                        )
                    )
                nc.sync.dma_start(out=pe[b0 : b0 + 128, lo:hi, :], in_=gt[:])

    nc.compile()
    return nc


def kernel(**inputs) -> tuple:
    text = np.ascontiguousarray(np.asarray(inputs["text"]).astype(np.int32))
    emb = np.ascontiguousarray(np.asarray(inputs["emb"], dtype=np.float32))
    ctx = np.ascontiguousarray(np.asarray(inputs["ctx"], dtype=np.float32))
    proj_w = np.ascontiguousarray(np.asarray(inputs["proj_w"], dtype=np.float32))
    proj_b = np.ascontiguousarray(
        np.asarray(inputs["proj_b"], dtype=np.float32).reshape(1, VD)
    )
    cpt = np.ascontiguousarray(np.asarray(inputs["cpt"], dtype=np.float32))
    cw = np.ascontiguousarray(np.asarray(inputs["cw"], dtype=np.float32))
    cb = np.ascontiguousarray(np.asarray(inputs["cb"], dtype=np.float32))

    if "nc" not in _CACHE:
        _CACHE["nc"] = _build_program()
    nc = _CACHE["nc"]

    in_maps = []
    for c in range(NCORES):
        in_maps.append(
            {
                "text": text[c * BS : (c + 1) * BS],
                "emb": emb,
                "ctx": ctx,
                "proj_w": proj_w,
                "proj_b": proj_b,
                "cpt": cpt,
                "cw": cw,
                "cb": cb,
            }
        )

    res = run_bass_kernel_spmd(nc, in_maps, core_ids=list(range(NCORES)))
    _CACHE["last_results"] = res

    pe = np.concatenate([res.results[c]["pe"] for c in range(NCORES)], axis=0)
    pt = np.concatenate([res.results[c]["pt"] for c in range(NCORES)], axis=0)
    proj_ctx = res.results[0]["proj_ctx"]
    vdp = res.results[0]["vdp"]
    return pe, pt, ctx.copy(), proj_ctx, cpt.copy(), vdp
